# revision 59
# baseline (speedup 1.0000x reference)
"""Trainium2 Bass kernel for nn_Angles2Coords (NeRF protein backbone placement).

Prefix product of rigid transforms (4x4 homogeneous, rows 0..2 = [R | t],
row3 = (0,0,0,1)):
  Z_0 = G0 @ T1(psi_0);  Z_i = T2(om_i) @ T3(ph_i) @ T1(ps_i)
  W_i = Z_0 ... Z_i;  residue i uses W_{i-1}: N = t, CA = R u2 + t, C = R u3 + t.

No-collective design (cross-core AllGather costs 30-50us on this stack):
every core redundantly computes the FULL 4096-residue scan (i = 32p + j on
[128 partitions, 32 j]); core 0's output is used.
  - one sin pass builds T1/T2/T3 (cos via sin(pi/2-|x|))
  - "Form-R" compose (rank-1 updates over m): all operands dense/broadcast
    (any stride-4-inner operand would run at 0.5x on DVE), row 3 and the
    affine column maintained by the compose itself
  - Kogge-Stone along j (5 levels), partition-exclusive scan via 7 PE
    shift-matmul rounds + identity fills
  - emission against the LOCAL prefix happens before the partition scan
    (fills the matmul ping-pong idle); the per-partition exclusive prefix
    E_p is applied afterwards via scalar_tensor_tensor with per-partition
    scalars, writing the x-major output directly.
No explicit drains between same-engine ops (HW pipe-flush preserves RAW
order; measured ~235ns saved per op); drains kept only under then_inc.
"""
import sys
import numpy as np

sys.path.insert(0, '/opt/trn_rl_repo')

import concourse.bass as bass
import concourse.mybir as mybir

F32 = mybir.dt.float32
NCORES = 8
L = 4096
J = 32
P = 128

R_N_CA = 1.458
R_CA_C = 1.525
R_C_N = 1.329
A_N_CA_C = np.deg2rad(111.2)
A_CA_C_N = np.deg2rad(116.2)
A_C_N_CA = np.deg2rad(121.7)

# T1 places N (psi), T2 places CA (omega), T3 places C (phi)
TH = {1: A_CA_C_N, 2: A_C_N_CA, 3: A_N_CA_C}
BB = {1: R_C_N, 2: R_N_CA, 3: R_CA_C}

# CONST columns (S-build consts replicated per j so (a,j) AP dims merge)
C_CA2 = 0                      # 3a x Jj x (3k x 2c)
C_CD = C_CA2 + 6 * 3 * J       # 3a x Jj x 3k
C_CB = C_CD + 3 * 3 * J        # 3a x Jj x 2k
C_HALFPI = C_CB + 2 * 3 * J
C_G0 = C_HALFPI + 4            # 16
C_ATOMS0T = C_G0 + 16          # 9, coord-major [x, a]
C_IDF = C_ATOMS0T + 9          # 128 = 8 levels x 16 identity fill
NCONST = C_IDF + 128


def _hom(R, t):
    H = np.zeros((4, 4))
    H[0:3, 0:3] = R
    H[0:3, 3] = t
    H[3, 3] = 1.0
    return H


def _g0h():
    N0 = np.zeros(3)
    CA0 = np.array([R_N_CA, 0.0, 0.0])
    aa = np.pi - A_N_CA_C
    C0 = CA0 + np.array([R_CA_C * np.cos(aa), R_CA_C * np.sin(aa), 0.0])
    bc = (C0 - CA0) / np.linalg.norm(C0 - CA0)
    n = np.cross(CA0 - N0, bc)
    n = n / np.linalg.norm(n)
    M0 = np.stack([bc, np.cross(n, bc), n], axis=1)
    return _hom(M0, C0), (N0, CA0, C0)


_IDH = _hom(np.eye(3), np.zeros(3))
KS_FILLS = [2, 1, 2, 4, 8, 16, 32, 64]  # merged excl+o1 round, then o=2..64


def make_consts():
    G0, (N0, CA0, C0) = _g0h()
    M = np.zeros((P, NCONST), dtype=np.float64)
    row = np.zeros(NCONST)
    for idx, a in enumerate((3, 1, 2)):          # atom order matches ang rows (phi,psi,omega)
        th, b = TH[a], BB[a]
        ct, st = np.cos(th), np.sin(th)
        ca2 = np.array([[-ct, -st], [st, -ct], [st, -ct]])        # [k, c]
        row[C_CA2 + 6 * J * idx: C_CA2 + 6 * J * (idx + 1)] = np.tile(
            ca2.reshape(-1), J)
        row[C_CD + 3 * J * idx: C_CD + 3 * J * (idx + 1)] = np.tile(
            np.array([-b * ct, b * st, b * st]), J)
        row[C_CB + 2 * J * idx: C_CB + 2 * J * (idx + 1)] = np.tile(
            np.array([-1.0, 1.0]), J)
    row[C_HALFPI] = np.pi / 2
    row[C_G0: C_G0 + 16] = G0.reshape(-1)
    row[C_ATOMS0T: C_ATOMS0T + 9] = np.stack([N0, CA0, C0], axis=1).reshape(-1)
    M[:] = row[None, :]
    # IDF varies by partition: level l fills identity for p < offset
    for lvl, off in enumerate(KS_FILLS):
        blk = np.zeros((P, 16))
        blk[0:off] = _IDH.reshape(-1)[None, :]
        M[:, C_IDF + 16 * lvl: C_IDF + 16 * lvl + 16] = blk
    return np.ascontiguousarray(M.astype(np.float32))


def make_shifts():
    # lhsT for shift-by-o: out[p] = in[p-o]  => lhsT[k, p] = 1 iff k = p-o
    mats = []
    for off in [1, 2, 4, 8, 16, 32, 64]:
        m = np.zeros((P, P), dtype=np.float32)
        m[np.arange(P - off), np.arange(off, P)] = 1.0
        mats.append(m)
    return np.concatenate(mats, axis=1)  # [128, 7*128]


def shard_angles(angles):
    sl = angles[0:3, :]
    a = np.ascontiguousarray(
        sl.reshape(3, P, J).transpose(1, 0, 2)).astype(np.float32).reshape(P, 3 * J)
    halfpi = np.full((P, 1), np.pi / 2, dtype=np.float32)
    return np.concatenate([a, halfpi], axis=1)   # [P, 3J+1]


def build_nc():
    nc = bass.Bass()
    ang = nc.declare_dram_parameter("ang", [P, 3 * J + 1], F32, isOutput=False)
    cst = nc.declare_dram_parameter("cst", [P, NCONST], F32, isOutput=False)
    shp = nc.declare_dram_parameter("shm", [P, 7 * P], F32, isOutput=False)
    out = nc.declare_dram_parameter("out", [P, 9 * J], F32, isOutput=True)

    AL = mybir.AluOpType
    AX = mybir.AxisListType
    AF = mybir.ActivationFunctionType

    from contextlib import ExitStack
    with ExitStack() as _es:
        def _e(cm):
            return _es.enter_context(cm)

        block = _e(nc.Block())
        dma_a = _e(nc.semaphore("dma_a"))
        dma_sh = _e(nc.semaphore("dma_sh"))
        g0s = _e(nc.semaphore("g0s"))
        trig_s = _e(nc.semaphore("trig_s"))
        v2pe = _e(nc.semaphore("v2pe"))
        pe2v = _e(nc.semaphore("pe2v"))
        fin_s = _e(nc.semaphore("fin_s"))

        ANG = _e(nc.sbuf_tensor("ANG", [P, 3 * J + 1], F32))
        CONST = _e(nc.sbuf_tensor("CONST", [P, NCONST], F32))
        SH = _e(nc.sbuf_tensor("SH", [P, 7 * P], F32))
        TRIG = _e(nc.sbuf_tensor("TRIG", [P, 9 * J], F32))   # [src3, ang3, J]
        BIGT = _e(nc.sbuf_tensor("BIGT", [P, 8 * 16 * J + 512], F32))
        U4 = _e(nc.sbuf_tensor("U4", [P, 12 * J], F32))      # [j, a, m4]
        OUTT = _e(nc.sbuf_tensor("OUTT", [P, 9 * J], F32))   # [x, j, a] coord-major
        PRD = _e(nc.sbuf_tensor("PRD", [P, 48 * J], F32))
        PSH = _e(nc.psum_tensor("PSH", [P, 16], F32))
        PSH2 = _e(nc.psum_tensor("PSH2", [P, 16], F32))

        UW = 16 * J   # big unit width

        def unit(u, nj=J):
            return BIGT[:, UW * u: UW * u + 16 * nj].rearrange(
                "p (j r c) -> p j r c", j=nj, r=4, c=4)

        def sunit(i):
            off = 8 * UW + 64 * i
            return BIGT[:, off: off + 16].rearrange(
                "p (j r c) -> p j r c", j=1, r=4, c=4)

        T3t, T1t, T2t = unit(0), unit(1), unit(2)
        P23v, Zv, WBv, WCv = unit(3), unit(4), unit(5), unit(6)
        CNDv, EPAv, EPBv, TOT0v = sunit(0), sunit(1), sunit(2), sunit(3)
        ALOC = BIGT[:, 7 * UW: 7 * UW + 9 * J]   # [m3, j, a] coord-major

        def prd3(pstart, pcount, w):
            """Three [p, w, 4, 4] scratch views in PRD."""
            return [PRD[pstart:pstart + pcount, 512 * i: 512 * i + 16 * w].rearrange(
                "p (j r c) -> p j r c", j=w, r=4, c=4) for i in range(3)]

        def compose(v, outv, Av, Bv, prds):
            """outv = Av @ Bv batched over j (XYZt affine; row3/col3 exact).
            Rank-1 over m=0..2 then column-3 fixup; all operands dense or
            broadcast (no transposed reads)."""
            Pp, w = outv.shape[0], outv.shape[1]
            p0, p1, p2 = prds
            for m in range(3):
                v.tensor_tensor(
                    out=[p0, p1, p2][m][:, 0:w, 0:3, :],
                    in0=Av[:, :, 0:3, m, None].broadcast_to([Pp, w, 3, 4]),
                    in1=Bv[:, :, m, None, :].broadcast_to([Pp, w, 3, 4]),
                    op=AL.mult)
            v.tensor_tensor(out=p0[:, 0:w, 0:3, :], in0=p0[:, 0:w, 0:3, :],
                            in1=p1[:, 0:w, 0:3, :], op=AL.add)
            v.tensor_tensor(out=outv[:, :, 0:3, :], in0=p0[:, 0:w, 0:3, :],
                            in1=p2[:, 0:w, 0:3, :], op=AL.add)
            v.tensor_tensor(out=outv[:, :, 0:3, 3], in0=outv[:, :, 0:3, 3],
                            in1=Av[:, :, 0:3, 3], op=AL.add)

        def compose_small(v, outv, Av, Bv, prd):
            """Drained compose for w == 1: one fused 48-element mult (all
            three output rows at once), then reduce. Sub-256-element
            dependent ops race even within the TENSOR_TENSOR class, hence
            the drains."""
            Pp = outv.shape[0]
            in1 = Bv.rearrange("p j m c -> p j c m")[:, 0, None, :, :]
            v.tensor_tensor(
                out=prd[:, 0],
                in0=Av[:, 0, 0:3, None, :].broadcast_to([Pp, 3, 4, 4]),
                in1=in1.broadcast_to([Pp, 3, 4, 4]), op=AL.mult)
            v.drain()
            v.tensor_reduce(
                out=outv[:, :, 0:3, :].rearrange("p j r c -> p j (r c)"),
                in_=prd.rearrange("p j r c m -> p j (r c) m"),
                axis=AX.X, op=AL.add)
            v.drain()

        def sprd(pstart, pcount, w):
            return PRD[pstart:pstart + pcount, 0:w * 48].rearrange(
                "p (j r c m) -> p j r c m", j=w, r=3, c=4, m=4)

        @block.sync
        def _(sync):
            sync.dma_start(out=ANG[:, :], in_=ang[:, :]).then_inc(dma_a, 16)
            sync.dma_start(out=CONST[:, :], in_=cst[:, :]).then_inc(dma_a, 16)
            sync.dma_start(out=SH[:, :], in_=shp[:, :]).then_inc(dma_sh, 16)
            sync.wait_ge(fin_s, 1)
            sync.dma_start(out=out[:, :], in_=OUTT[:, :]).then_inc(dma_sh, 16)

        @block.gpsimd
        def _(g):
            z3 = BIGT[:, 0:7 * UW].rearrange("p (u j r c) -> p u j r c",
                                             u=7, j=J, r=4, c=4)
            g.memset(z3[:, :, :, 3:4, 0:3], 0.0).then_inc(g0s, 1)
            g.memset(z3[:, :, :, 3:4, 3:4], 1.0).then_inc(g0s, 1)
            z3s = BIGT[:, 8 * UW: 8 * UW + 512].rearrange(
                "p (u j r c) -> p u j r c", u=32, j=1, r=4, c=4)
            g.memset(z3s[:, :, :, 3:4, 0:3], 0.0).then_inc(g0s, 1)
            g.memset(z3s[:, :, :, 3:4, 3:4], 1.0).then_inc(g0s, 1)
            g.memset(TRIG[:, 0:3 * J], 1.0).then_inc(g0s, 1)
            u4v = U4[:, :].rearrange("p (j a m) -> p j a m", j=J, a=3, m=4)
            g.memset(U4[:, :], 0.0).then_inc(g0s, 1)
            g.drain()
            g.memset(u4v[:, :, :, 3:4], 1.0).then_inc(g0s, 1)

        @block.scalar
        def _(s):
            # prefetch the sin + abs activation tables while input DMAs run
            s.activation(PRD[:, 0:4], PRD[:, 0:4], AF.Sin)
            s.activation(PRD[:, 4:8], PRD[:, 4:8], AF.Abs)
            s.wait_ge(dma_a, 16)     # ANG only (pi/2 rides in its last col)
            s.activation(TRIG[:, 6 * J:9 * J], ANG[:, 0:3 * J], AF.Sin)
            s.activation(TRIG[:, 3 * J:6 * J], ANG[:, 0:3 * J], AF.Abs)
            s.drain()
            s.activation(TRIG[:, 3 * J:6 * J], TRIG[:, 3 * J:6 * J], AF.Sin,
                         bias=ANG[:, 3 * J:3 * J + 1], scale=-1.0)
            s.drain().then_inc(trig_s, 1)

        @block.tensor
        def _(t):
            t.wait_ge(dma_sh, 16)
            t.wait_ge(v2pe, 1)
            tot_f = TOT0v[:, 0, :, :].rearrange("p r c -> p (r c)")
            t.matmul(PSH[:, :], SH[:, 0:P], tot_f, start=True, stop=True)
            t.matmul(PSH2[:, :], SH[:, P:2 * P], tot_f,
                     start=True, stop=True).then_inc(pe2v, 1)
            for lvl, off in enumerate([2, 4, 8, 16, 32, 64]):
                oi = [1, 2, 4, 8, 16, 32, 64].index(off)
                t.wait_ge(v2pe, lvl + 2)
                src = EPAv if lvl % 2 == 0 else EPBv
                t.matmul(PSH[:, :], SH[:, P * oi: P * (oi + 1)],
                         src[:, 0, :, :].rearrange("p r c -> p (r c)"),
                         start=True, stop=True).then_inc(pe2v, 1)

        @block.vector
        def _(v):
            # S-build: merged (a j) views: TS as [p, aj, 4, 4]; trig [p, s, aj]
            G3 = 3 * J
            TSm = BIGT[:, 0:3 * UW].rearrange("p (g r c) -> p g r c",
                                              g=G3, r=4, c=4)
            trig_m = TRIG[:, :].rearrange("p (s g) -> p s g", s=3, g=G3)
            tbm = trig_m.rearrange("p s g -> p g s")
            v.memset(TSm[:, :, 0:1, 2:3], 0.0)   # S[0, z] = 0 (pre-wait:
            # no interlock between MEMSET and the TENSOR_TENSOR class)
            v.wait_ge(trig_s, 1)
            v.wait_ge(g0s, 7)
            v.wait_ge(dma_a, 32)     # CONST (S-build coefficient tables)
            ca2 = CONST[:, C_CA2:C_CA2 + 6 * G3].rearrange(
                "p (g k c) -> p g k c", g=G3, k=3, c=2)
            v.tensor_tensor(out=TSm[:, :, 0:3, 0:2],
                            in0=tbm[:, :, :, None].broadcast_to([P, G3, 3, 2]),
                            in1=ca2, op=AL.mult)
            cd = CONST[:, C_CD:C_CD + 3 * G3].rearrange("p (g k) -> p g k",
                                                        g=G3, k=3)
            v.tensor_tensor(out=TSm[:, :, 0:3, 3:4],
                            in0=tbm[:, :, :, None],
                            in1=cd[:, :, :, None], op=AL.mult)
            cb = CONST[:, C_CB:C_CB + 2 * G3].rearrange("p (g k) -> p g k",
                                                        g=G3, k=2)
            v.tensor_tensor(out=TSm[:, :, 1:2, 2:3],
                            in0=tbm[:, :, 2:3, None],
                            in1=cb[:, :, 0:1, None], op=AL.mult)
            v.tensor_tensor(out=TSm[:, :, 2:3, 2:3],
                            in0=tbm[:, :, 1:2, None],
                            in1=cb[:, :, 1:2, None], op=AL.mult)
            v.drain()   # the first compose reads T entries written just above

            # P23 = T2 o T3 ; Z = P23 o T1
            compose(v, P23v[:, :], T2t, T3t, prd3(0, P, J))
            compose(v, Zv[:, :], P23v, T1t, prd3(0, P, J))

            # seed: Z[0,0] = G0 o T1[0,0]  (every core runs the full chain)
            G0v = CONST[:, C_G0:C_G0 + 16].rearrange("p (r c) -> p r c",
                                                     r=4, c=4)
            compose_small(v, Zv[0:1, 0:1], G0v[0:1, None, :, :],
                          T1t[0:1, 0:1], sprd(0, 1, 1))

            # u4 assembly early (COPY class: keep far from its readers)
            u4v = U4[:, :].rearrange("p (j a m) -> p j a m", j=J, a=3, m=4)
            v.tensor_copy(out=u4v[:, :, 1:2, 0:3],
                          in_=T2t[:, :, None, 0:3, 3])
            v.tensor_copy(out=u4v[:, :, 2:3, 0:3],
                          in_=P23v[:, :, None, 0:3, 3])

            # Pair-fold Kogge-Stone: fold residue pairs (Z2[q] = Z[2q] o
            # Z[2q+1]), scan the 16 pair-products in 4 levels, then recover
            # even-slot prefixes with one more compose. 80 slot-composes
            # instead of 129.
            H = J // 2
            rprd = PRD[:, 24 * 48: 25 * 48].rearrange(
                "p (j r c m) -> p j r c m", j=1, r=3, c=4, m=4)

            def hunit(u):
                return BIGT[:, UW * u: UW * u + 16 * H].rearrange(
                    "p (j r c) -> p j r c", j=H, r=4, c=4)

            h5, h6, h0 = hunit(5), hunit(6), hunit(0)
            compose(v, h5[:, :], Zv[:, 0:J:2], Zv[:, 1:J:2], prd3(0, P, H))
            ks_seq = [(h5, h6), (h6, h0), (h0, h5), (h5, h6)]
            for lvl, (sv, dv) in enumerate(ks_seq):
                o = 1 << lvl
                su = [5, 6, 0, 5][lvl]
                du = [6, 0, 5, 6][lvl]
                v.tensor_copy(out=BIGT[:, UW * du: UW * du + 16 * o],
                              in_=BIGT[:, UW * su: UW * su + 16 * o])
                if lvl == 3:
                    v.drain()   # guard: 96-element dependent ops below
                compose(v, dv[:, o:H], sv[:, 0:H - o], sv[:, o:H],
                        prd3(0, P, H - o))
                if lvl == 2:
                    # TOT = s7 o s15 of the window-8 level: releases the
                    # tensor engine two levels early
                    v.drain()
                    compose_small(v, TOT0v[:, :], dv[:, 7:8], dv[:, 15:16],
                                  rprd)
                    v.drain().then_inc(v2pe, 1)
            S2 = h6          # S2[q] = W[2q+1], q = 0..15
            # evens: Wev[q] = W[2q]; Wev[0] = Z[0], Wev[q>=1] = S2[q-1] o Z[2q]
            Wev = h5
            v.tensor_copy(out=BIGT[:, UW * 5: UW * 5 + 16],
                          in_=BIGT[:, UW * 4: UW * 4 + 16])
            v.drain()   # reads S2 slots the lvl-3 compose just wrote
            compose(v, Wev[:, 1:H], S2[:, 0:H - 1], Zv[:, 2:J:2],
                    prd3(0, P, H - 1))

            # local emission pieces, interleaved into the p-scan ping-pong
            # idle: aloc[m, j, a] = sum_k W[j-1][m, k] * u4[j, a, k] (j>=1);
            # odd j reads Wev, even j reads S2
            eprd = PRD[:, 0:36 * J].rearrange("p (m j a k) -> p m j a k",
                                              m=3, j=J, a=3, k=4)
            alocv = ALOC.rearrange("p (m j a) -> p m j a", m=3, j=J, a=3)

            def epiece(k):
                if k < 3:
                    v.tensor_tensor(
                        out=eprd[:, k, 1:J:2],
                        in0=Wev[:, 0:H, None, k, :].broadcast_to(
                            [P, H, 3, 4]),
                        in1=u4v[:, 1:J:2], op=AL.mult)
                    v.tensor_tensor(
                        out=eprd[:, k, 2:J:2],
                        in0=S2[:, 0:H - 1, None, k, :].broadcast_to(
                            [P, H - 1, 3, 4]),
                        in1=u4v[:, 2:J:2], op=AL.mult)
                elif k < 6:
                    m = k - 3
                    if k == 3:
                        v.drain()   # reduce class vs the eprd mults
                    v.tensor_reduce(
                        out=ALOC[:, 3 * J * m: 3 * J * (m + 1)],
                        in_=eprd[:, m].rearrange("p j a k -> p (j a) k"),
                        axis=AX.X, op=AL.add)
                elif k == 6:
                    v.drain()   # WAW: overwrites reduce output at j=0
                    v.tensor_copy(
                        out=alocv[:, :, 0, :],
                        in_=u4v[:, 0, :, 0:3].rearrange("p a m -> p m a"))

            # partition-exclusive prefixes: merged excl+o1 round, then o=2..64
            # (round scratch lives past the emission buffer in PRD)
            epiece(0)    # fills the gap while the first matmuls run
            v.wait_ge(pe2v, 1)
            idfA = CONST[:, C_IDF: C_IDF + 16]
            idfB = CONST[:, C_IDF + 16: C_IDF + 32]
            v.tensor_tensor(out=BIGT[:, 8 * UW + 2 * 64: 8 * UW + 2 * 64 + 16],
                            in0=PSH[:, :], in1=idfB, op=AL.add)
            v.tensor_tensor(out=BIGT[:, 8 * UW: 8 * UW + 16],
                            in0=PSH2[:, :], in1=idfA, op=AL.add)
            v.drain()   # 16-elem ops: compose reads CND/EPB just written
            compose_small(v, EPAv[:, :], CNDv, EPBv, rprd)
            v.drain().then_inc(v2pe, 1)
            epiece(1)
            PSHv = PSH[:, :].rearrange("p (j r c) -> p j r c", j=1, r=4, c=4)
            cur = EPAv
            for lvl, off in enumerate([2, 4, 8, 16, 32, 64]):
                v.wait_ge(pe2v, lvl + 2)
                if off == 64:
                    # base-32/64 partition slices are legal: compose in
                    # place on the top rows, no identity fill needed
                    rpo = PRD[off:P, 24 * 48: 25 * 48].rearrange(
                        "p (j r c m) -> p j r c m", j=1, r=3, c=4, m=4)
                    compose_small(v, cur[off:P, :], PSHv[off:P],
                                  cur[off:P, :], rpo)
                    nxt = cur
                else:
                    idf = CONST[:, C_IDF + 16 * (lvl + 2):
                                C_IDF + 16 * (lvl + 2) + 16]
                    v.tensor_tensor(out=BIGT[:, 8 * UW: 8 * UW + 16],
                                    in0=PSH[:, :], in1=idf, op=AL.add)
                    v.drain()
                    nxt = EPBv if cur is EPAv else EPAv
                    compose_small(v, nxt[:, :], CNDv, cur, rprd)
                if lvl < 5:
                    v.drain().then_inc(v2pe, 1)
                epiece(lvl + 2)
                cur = nxt
            epbase = 8 * UW + 64 * (1 if cur is EPAv else 2)

            # final apply: OUT[x, (j a)] = sum_m E_p[x, m] * aloc[m] + E_p[x, 3]
            # all TENSOR_TENSOR with stride-0 broadcasts of the E elements
            v.drain()   # mults read the EP slot the last compose just wrote
            E16 = BIGT[:, epbase:epbase + 16].rearrange("p (x m) -> p x m",
                                                        x=4, m=4)
            G = 3 * J
            pm = PRD[:, 0:9 * G].rearrange("p (x m g) -> p x m g",
                                           x=3, m=3, g=G)
            alc = ALOC[:, :].rearrange("p (m g) -> p m g", m=3, g=G)
            v.tensor_tensor(
                out=pm[:, :, :, :],
                in0=E16[:, 0:3, 0:3, None].broadcast_to([P, 3, 3, G]),
                in1=alc[:, None, :, :].broadcast_to([P, 3, 3, G]),
                op=AL.mult)
            OV = OUTT[:, :].rearrange("p (x g) -> p x g", x=3, g=G)
            v.tensor_tensor(out=pm[:, :, 0, :], in0=pm[:, :, 0, :],
                            in1=pm[:, :, 1, :], op=AL.add)
            v.tensor_tensor(out=pm[:, :, 0, :], in0=pm[:, :, 0, :],
                            in1=pm[:, :, 2, :], op=AL.add)
            v.tensor_tensor(out=OV[:, :, :], in0=pm[:, :, 0, :],
                            in1=E16[:, 0:3, 3:4].broadcast_to([P, 3, G]),
                            op=AL.add)
            # (residue 0 is a compile-time constant, patched host-side)
            v.drain().then_inc(fin_s, 1)

    return nc


_CACHED = {}


def make_in_maps(angles):
    if "nc" not in _CACHED:
        _CACHED["nc"] = build_nc()
        _CACHED["sh"] = make_shifts()
        _CACHED["cst"] = make_consts()
    a = shard_angles(angles)
    return [{
        "ang": a,
        "cst": _CACHED["cst"],
        "shm": _CACHED["sh"],
    } for _ in range(NCORES)]


def kernel(input_angles):
    angles = np.asarray(input_angles, dtype=np.float32)
    in_maps = make_in_maps(angles)
    nc = _CACHED["nc"]
    from concourse.bass_utils import run_bass_kernel_spmd
    res = run_bass_kernel_spmd(nc, in_maps, list(range(NCORES)))
    o = res.results[0]["out"].reshape(P, 3, J, 3)       # (p, x, j, a)
    full = np.ascontiguousarray(
        o.transpose(0, 2, 3, 1).reshape(L, 3, 3).astype(np.float32))
    _, (N0, CA0, C0) = _g0h()
    full[0] = np.stack([N0, CA0, C0]).astype(np.float32)
    return full.reshape(-1)
